# revision 16
# baseline (speedup 1.0000x reference)
"""Trainium2 Bass kernel for BlankEmbedding (embedding lookup + blank shift-accumulate).

Reference semantics:
    out = emb[x]                                    # [B, S, D] gather
    preblank[p] = (x[p+1]==BLANK) & (x[p]!=BLANK)   (per row; zero-padded shifts)
    out[p+k] += preblank[p] * emb[x[p]]  for k in 1..3

Strategy: data-parallel over the 16384 flattened tokens, 2048 per core.
The device gathers int8-quantized rows (global absmax/127 scale; ~7.8e-3
rel err vs the 2e-2 budget) and stores them unmodified; the host applies
the scale while unsharding. Sparse blank fixups (P(blank)=1/50257) are
recomputed on-device in int16 and placed by the host.

- Gathers run on the SWDGE indirect-DMA path: descgen is the bottleneck
  (~1.1us per 128-row instruction, engine-serial; measured that neither
  multiple SWDGE queues nor InstDMAGatherAnt beat it once its ~9us mlp
  ucode library load is accounted). Layout ix[p, j] = token 16p + j, so
  each partition holds 16 consecutive tokens and each store descriptor
  is contiguous in DRAM.
- int8 end-to-end halves both the random-row reads (1KB rows) and the
  store traffic vs the bf16 variant, and removes the DVE dequant stage.
- The two fixup gathers sit right after the first main gather so their
  adds + fixout store complete under the main chain instead of tailing
  it. Unused fixup slots read the appended zero row (index VOCAB).
"""

import numpy as np

VOCAB = 50257
ZROW = VOCAB                 # appended all-zeros table row (no-op addend)
DIM = 1024
BLANK = 100
N_BLANKS = 3
B, S = 4, 4096
N_CORES = 8
TOK = B * S                  # 16384 flattened tokens
TPC = TOK // N_CORES         # 2048 tokens per core
P = 128                      # SBUF partitions
NJ = TPC // P                # 16 tokens per partition

_CACHE = {}


def _build_nc(kfix=16, has2=False):
    from concourse import bacc, mybir, tile
    import concourse.bass as bass

    nc = bacc.Bacc(
        "TRN2", target_bir_lowering=False, debug=False, num_devices=1
    )
    i8 = mybir.dt.int8
    i16 = mybir.dt.int16
    i32 = mybir.dt.int32

    ix_dram = nc.dram_tensor("ix", [P, NJ], i32, kind="ExternalInput")
    emb8 = nc.dram_tensor("emb8", [VOCAB + 1, DIM], i8, kind="ExternalInput")
    fix_dram = nc.dram_tensor("fix", [P, 3], i32, kind="ExternalInput")
    out = nc.dram_tensor("out", [TPC, DIM], i8, kind="ExternalOutput")
    fixout = nc.dram_tensor("fixout", [kfix, DIM], i16, kind="ExternalOutput")

    with tile.TileContext(nc) as tc:
        with tc.tile_pool(name="sbuf", bufs=1) as pool:
            ix_all = pool.tile([P, NJ], i32)
            fix_sb = pool.tile([P, 3], i32)  # cols: xt, s1, s2
            nc.sync.dma_start(out=ix_all[:], in_=ix_dram[:])
            nc.scalar.dma_start(out=fix_sb[:], in_=fix_dram[:])

            g8 = pool.tile([P, NJ * DIM], i8)
            out3 = out[:].rearrange("(p j) d -> p j d", p=P, j=NJ)

            # out[16p+j] = g8[p, j*DIM:...]: columns j..j+k of a partition
            # are contiguous in DRAM, so grouped stores use few big
            # descriptors. Front groups are big (descgen-chain slack); the
            # last is a single column to minimize the post-chain tail.
            store_after = {5: (0, 6), 10: (6, 11), 14: (11, 15), 15: (15, 16)}

            def main_gather(j):
                nc.gpsimd.indirect_dma_start(
                    out=g8[:, j * DIM : (j + 1) * DIM],
                    out_offset=None,
                    in_=emb8[:],
                    in_offset=bass.IndirectOffsetOnAxis(
                        ap=ix_all[:, j : j + 1], axis=0
                    ),
                )
                if j in store_after:
                    c0, c1 = store_after[j]
                    nc.sync.dma_start(
                        out=out3[:, c0:c1, :],
                        in_=g8[:, c0 * DIM : c1 * DIM],
                    )

            # fixup gathers ride second/third in the descgen chain so the
            # whole fixup path finishes under the main chain's shadow
            main_gather(0)
            ab = pool.tile([P, DIM], i8)
            a1 = pool.tile([P, DIM], i8)
            cols = ((ab, 0), (a1, 1))
            if has2:
                a2 = pool.tile([P, DIM], i8)
                cols += ((a2, 2),)
            for t, col in cols:
                nc.gpsimd.indirect_dma_start(
                    out=t[:kfix, :], out_offset=None, in_=emb8[:],
                    in_offset=bass.IndirectOffsetOnAxis(
                        ap=fix_sb[:kfix, col : col + 1], axis=0
                    ),
                )
            for j in range(1, NJ):
                main_gather(j)

            # fixout[k] = emb8[xt_k] + emb8[s1_k] (+ emb8[s2_k]) in int16
            w0 = pool.tile([P, DIM], i16)
            w1 = pool.tile([P, DIM], i16)
            wide = [w0, w1]
            if has2:
                w2 = pool.tile([P, DIM], i16)
                wide.append(w2)
            for (t, _), w in zip(cols, wide):
                nc.vector.tensor_scalar(
                    out=w[:kfix, :], in0=t[:kfix, :],
                    scalar1=1.0, scalar2=None, op0=mybir.AluOpType.mult,
                )
            if has2:
                nc.vector.tensor_tensor(
                    out=w1[:kfix, :], in0=w1[:kfix, :], in1=w2[:kfix, :],
                    op=mybir.AluOpType.add,
                )
            nc.vector.tensor_tensor(
                out=w0[:kfix, :], in0=w0[:kfix, :], in1=w1[:kfix, :],
                op=mybir.AluOpType.add,
            )
            nc.scalar.dma_start(out=fixout[:], in_=w0[:kfix, :])

    nc.compile()
    return nc


def get_nc(kfix=16, has2=False):
    key = (kfix, has2)
    if key not in _CACHE:
        _CACHE[key] = _build_nc(kfix, has2)
    return _CACHE[key]


def _corrections(x2):
    """Exact reference semantics: list of (global_target_row, src_token)."""
    is_blank = x2 == BLANK
    prev = np.zeros_like(is_blank)
    prev[:, 1:] = is_blank[:, :-1]
    first_blank = is_blank & ~prev
    out = []
    for b, f in np.argwhere(first_blank):
        if f == 0:
            continue  # run at row start: reference shifts in zeros
        p = f - 1
        src_tok = int(x2[b, p])
        for k in range(1, N_BLANKS + 1):
            s = p + k
            if s >= S:
                break
            out.append((b * S + s, src_tok))
    return out


def shard_inputs(x, emb_table):
    """Returns (in_maps, fix_targets, kfix, has2, scale); fix_targets[c]
    maps fixout slot -> core-local target row."""
    x2 = np.asarray(x).astype(np.int64).reshape(B, S)
    flat = x2.reshape(-1).astype(np.int32)
    emb_f = np.asarray(emb_table, dtype=np.float32)
    scale = float(np.abs(emb_f).max()) / 127.0
    emb_i8 = np.vstack(
        [
            np.clip(np.rint(emb_f / scale), -127, 127).astype(np.int8),
            np.zeros((1, DIM), dtype=np.int8),
        ]
    )

    # per-target slots: tgt -> up to 2 src tokens (two blank runs can land
    # on one target only at distance 2; adjacent first-blanks are impossible)
    per_tgt = {}
    for tgt, src in _corrections(x2):
        per_tgt.setdefault(tgt, []).append(src)
    assert all(len(v) <= 2 for v in per_tgt.values()), per_tgt
    has2 = any(len(v) > 1 for v in per_tgt.values())
    max_per_core = max(
        sum(1 for t in per_tgt if c * TPC <= t < (c + 1) * TPC)
        for c in range(N_CORES)
    )
    kfix = 16 if max_per_core <= 16 else P

    in_maps = []
    fix_targets = []
    for c in range(N_CORES):
        base = c * TPC
        ix = np.ascontiguousarray(flat[base : base + TPC].reshape(P, NJ))

        fix = np.full((P, 3), ZROW, dtype=np.int32)  # xt, s1, s2
        fix[:, 0] = 0  # unused slots recompute emb[0]+0+0; host ignores them
        mine = {t: v for t, v in per_tgt.items() if base <= t < base + TPC}
        assert len(mine) <= kfix, "fixup slot overflow"
        targets = {}
        for slot, (t, srcs) in enumerate(mine.items()):
            fix[slot] = [flat[t], srcs[0], srcs[1] if len(srcs) > 1 else ZROW]
            targets[slot] = t - base
        fix_targets.append(targets)
        in_maps.append({"ix": ix, "emb8": emb_i8, "fix": fix})
    return in_maps, fix_targets, kfix, has2, scale


def assemble_output(results, fix_targets, scale):
    parts = []
    for c in range(N_CORES):
        part = results[c]["out"].astype(np.float32) * scale
        targets = fix_targets[c]
        if targets:
            fo = results[c]["fixout"]
            for slot, loc in targets.items():
                part[loc] = fo[slot].astype(np.float32) * scale
        parts.append(part)
    return np.concatenate(parts, axis=0).reshape(B, S, DIM)


def kernel(x, emb_table):
    from concourse.bass_utils import run_bass_kernel_spmd

    in_maps, fix_targets, kfix, has2, scale = shard_inputs(x, emb_table)
    nc = get_nc(kfix, has2)
    res = run_bass_kernel_spmd(nc, in_maps, core_ids=list(range(N_CORES)))
    return assemble_output(res.results, fix_targets, scale)
